# revision 4
# baseline (speedup 1.0000x reference)
"""Differential attention Trainium2 kernel (Bass/Tile), 8-core data parallel.

Sharding: core c handles batch b = c//2 and query half h = c%2.
Each core receives x[b]^T (bf16, host-transposed; key order rolled by 2048
for odd cores so "my queries" are always columns 0:2048), computes
K^T/Q^T/V projections on-chip, then per 128-query tile:
  A1/A2 logits (row-packed 64-contraction matmuls) -> PSUM
  exp via ScalarE (scale=1/8 folded in, accum_out gives row sums free)
  combined = exp1/s1 - lambda*exp2/s2 via 2 DVE ops (per-partition scalars)
  combined -> DRAM (SWDGE bf16->fp32 cast DMA)
  combined^T via xbar DMA transpose -> AV matmul (32 accumulating MMs)
Softmax max-subtraction is skipped: logits are ~N(0, 0.33), |logit| < ~6,
exp is far from overflow and jax.nn.softmax(x) == softmax(x - max) exactly
up to fp rounding.
"""

import math
import sys

sys.path.insert(0, "/opt/trn_rl_repo")

import ml_dtypes
import numpy as np

B, S, D = 4, 4096, 1024
HD = 64
E = 2 * HD  # 128
P = 128
DEPTH = 12
NCORES = 8

_BUILD_CACHE: dict = {}


def _build(lam: float, s: int, d: int, qh: int, n_iter: int = 1):
    """Build + bacc-compile the single-core program (shared by all 8 cores).

    s: keys per core, d: model dim, qh: queries per core.
    n_iter: repeat the whole compute n_iter times (for timing deltas).
    """
    from contextlib import ExitStack

    import concourse.mybir as mybir
    import concourse.tile as tile
    from concourse import bacc
    from concourse.bass import ds, ts

    F32 = mybir.dt.float32
    BF16 = mybir.dt.bfloat16
    AF = mybir.ActivationFunctionType
    OP = mybir.AluOpType

    DC = d // P          # d-chunks
    NKT = s // P         # 128-wide key tiles (32)
    NQT = qh // P        # 128-query tiles (16)
    KHALF = s // 2       # 2048
    ACW = min(512, KHALF)  # A-matmul chunk width
    NKC = KHALF // ACW   # chunks per half
    PCW = min(512, s)    # projection chunk width
    scale = HD ** -0.5

    nc = bacc.Bacc(
        "TRN2",
        target_bir_lowering=False,
        debug=False,
        enable_asserts=False,
        num_devices=1,
    )
    xt_d = nc.dram_tensor("xt", [d, s], BF16, kind="ExternalInput")
    wq_d = nc.dram_tensor("wqt", [d, E], BF16, kind="ExternalInput")
    wk_d = nc.dram_tensor("wkt", [d, E], BF16, kind="ExternalInput")
    wv_d = nc.dram_tensor("wvt", [d, E], BF16, kind="ExternalInput")
    comb_d = nc.dram_tensor("comb", [qh, s], F32, kind="ExternalOutput")
    out_d = nc.dram_tensor("out", [qh, E], F32, kind="ExternalOutput")

    with tile.TileContext(nc) as tc, ExitStack() as ctx:
        const = ctx.enter_context(tc.tile_pool(name="const", bufs=1))
        psum = ctx.enter_context(tc.tile_pool(name="psum", bufs=2, space="PSUM"))

        wq = const.tile([P, DC, E], BF16, tag="wq")
        wk = const.tile([P, DC, E], BF16, tag="wk")
        wv = const.tile([P, DC, E], BF16, tag="wv")
        nc.sync.dma_start(wq[:], wq_d.ap().rearrange("(c p) e -> p c e", p=P))
        nc.sync.dma_start(wk[:], wk_d.ap().rearrange("(c p) e -> p c e", p=P))
        nc.sync.dma_start(wv[:], wv_d.ap().rearrange("(c p) e -> p c e", p=P))

        KT = const.tile([P, s], BF16, tag="KT")     # [e, k]; K1^T rows 0:64, K2^T rows 64:128
        QT = const.tile([P, qh], BF16, tag="QT")    # [e, q]
        V = const.tile([P, NKT, E], BF16, tag="V")  # [k%128, k//128, e]
        outbuf = const.tile([P, NQT, E], F32, tag="outbuf")

        for it in range(n_iter):
            with tc.tile_pool(name=f"xtp{it}", bufs=1) as xtp:
                xt = xtp.tile([P, DC, s], BF16, tag="xt")
                xt_re = xt_d.ap().rearrange("(c p) s -> p c s", p=P)
                NSC = s // PCW
                for sc in range(NSC):
                    nc.sync.dma_start(xt[:, :, ts(sc, PCW)], xt_re[:, :, ts(sc, PCW)])

                # K^T chunks: out[e, s_chunk]; lhsT = W^T chunk, rhs = x^T chunk
                for sc in range(NSC):
                    ps = psum.tile([P, PCW], F32, tag="ps")
                    for dc in range(DC):
                        nc.tensor.matmul(
                            ps[:], wk[:, dc, :], xt[:, dc, ts(sc, PCW)],
                            start=(dc == 0), stop=(dc == DC - 1),
                        )
                    nc.vector.tensor_copy(KT[:, ts(sc, PCW)], ps[:])
                QCW = min(512, qh)
                for sc in range(qh // QCW):
                    ps = psum.tile([P, QCW], F32, tag="ps")
                    for dc in range(DC):
                        nc.tensor.matmul(
                            ps[:], wq[:, dc, :], xt[:, dc, ts(sc, QCW)],
                            start=(dc == 0), stop=(dc == DC - 1),
                        )
                    nc.vector.tensor_copy(QT[:, ts(sc, QCW)], ps[:])
                # V natural [k, e]: lhsT = x^T k-tile (stationary), rhs = Wv^T chunk
                for kt in range(NKT):
                    ps = psum.tile([P, E], F32, tag="ps")
                    for dc in range(DC):
                        nc.tensor.matmul(
                            ps[:], xt[:, dc, ts(kt, P)], wv[:, dc, :],
                            start=(dc == 0), stop=(dc == DC - 1),
                        )
                    nc.vector.tensor_copy(V[:, kt, :], ps[:])

            with (
                tc.tile_pool(name=f"work{it}", bufs=2) as work,
                tc.tile_pool(name=f"scratch{it}", bufs=1) as scratch,
            ):
                for qt in range(NQT):
                    exp1 = work.tile([P, s], BF16, tag="exp1")
                    exp2 = work.tile([P, s], BF16, tag="exp2")
                    sp = work.tile([P, 4], F32, tag="sp")
                    for half in range(2):
                        psA1 = psum.tile([P, KHALF], F32, tag="ps")
                        psA2 = psum.tile([P, KHALF], F32, tag="ps")
                        for kc in range(NKC):
                            koff = half * KHALF + kc * ACW
                            nc.tensor.matmul(
                                psA1[:, ts(kc, ACW)],
                                QT[0:64, ts(qt, P)], KT[0:64, ds(koff, ACW)],
                                start=True, stop=True,
                            )
                            nc.tensor.matmul(
                                psA2[:, ts(kc, ACW)],
                                QT[64:128, ts(qt, P)], KT[64:128, ds(koff, ACW)],
                                start=True, stop=True,
                            )
                        nc.scalar.activation(
                            exp1[:, ds(half * KHALF, KHALF)], psA1[:], AF.Exp,
                            scale=scale, accum_out=sp[:, 0 + half : 1 + half],
                        )
                        nc.scalar.activation(
                            exp2[:, ds(half * KHALF, KHALF)], psA2[:], AF.Exp,
                            scale=scale, accum_out=sp[:, 2 + half : 3 + half],
                        )
                    s12 = work.tile([P, 2], F32, tag="s12")
                    nc.vector.tensor_tensor(s12[:, 0:1], sp[:, 0:1], sp[:, 1:2], OP.add)
                    nc.vector.tensor_tensor(s12[:, 1:2], sp[:, 2:3], sp[:, 3:4], OP.add)
                    r12 = work.tile([P, 2], F32, tag="r12")
                    nc.vector.reciprocal(r12[:], s12[:])
                    r2l = work.tile([P, 1], F32, tag="r2l")
                    nc.vector.tensor_scalar_mul(r2l[:], r12[:, 1:2], lam)

                    tmp = scratch.tile([P, s], BF16, tag="tmp")
                    nc.vector.tensor_scalar_mul(tmp[:], exp2[:], r2l[:, 0:1])
                    comb = work.tile([P, s], BF16, tag="comb")
                    nc.vector.scalar_tensor_tensor(
                        comb[:], exp1[:], r12[:, 0:1], tmp[:], OP.mult, OP.subtract
                    )

                    nc.gpsimd.dma_start(comb_d.ap()[ds(qt * P, P), :], comb[:])

                    cT = work.tile([P, NKT, P], BF16, tag="cT")
                    nc.sync.dma_start_transpose(cT[:], comb[:])
                    psO = psum.tile([P, E], F32, tag="ps")
                    for c in range(NKT):
                        nc.tensor.matmul(
                            psO[:], cT[:, c, :], V[:, c, :],
                            start=(c == 0), stop=(c == NKT - 1),
                        )
                    nc.vector.tensor_copy(outbuf[:, qt, :], psO[:])

        nc.sync.dma_start(out_d.ap().rearrange("(t p) e -> p t e", p=P), outbuf[:])

    nc.compile()
    return nc


def _get_nc(lam: float, n_iter: int = 1):
    key = (round(lam, 6), n_iter)
    if key not in _BUILD_CACHE:
        _BUILD_CACHE[key] = _build(lam, S, D, S // 2, n_iter)
    return _BUILD_CACHE[key]


def _prep_inputs(x, Wq, Wk, Wv):
    bf = ml_dtypes.bfloat16
    x = np.asarray(x, dtype=np.float32)
    wqt = np.ascontiguousarray(np.asarray(Wq, np.float32).T).astype(bf)
    wkt = np.ascontiguousarray(np.asarray(Wk, np.float32).T).astype(bf)
    wvt = np.ascontiguousarray(np.asarray(Wv, np.float32).T).astype(bf)
    roll = np.r_[S // 2 : S, 0 : S // 2]
    in_maps = []
    for c in range(NCORES):
        b, h = divmod(c, 2)
        xb = x[b] if h == 0 else x[b][roll]
        xt = np.ascontiguousarray(xb.T).astype(bf)
        in_maps.append({"xt": xt, "wqt": wqt, "wkt": wkt, "wvt": wvt})
    return in_maps, roll


def kernel(x, Wq, Wk, Wv, lambda_q1, lambda_q2, lambda_k1, lambda_k2):
    from concourse.bass_utils import run_bass_kernel_spmd

    lq1 = np.asarray(lambda_q1, np.float64)
    lq2 = np.asarray(lambda_q2, np.float64)
    lk1 = np.asarray(lambda_k1, np.float64)
    lk2 = np.asarray(lambda_k2, np.float64)
    lam_init = 0.8 - 0.6 * math.exp(-0.3 * DEPTH)
    lam = float(
        np.exp(np.sum(lq1 * lk1)) - np.exp(np.sum(lq2 * lk2)) + lam_init
    )

    nc = _get_nc(lam)
    in_maps, roll = _prep_inputs(x, Wq, Wk, Wv)
    res = run_bass_kernel_spmd(nc, in_maps, core_ids=list(range(NCORES)))

    QH = S // 2
    combined = np.empty((B, S, S), np.float32)
    output = np.empty((B, S, E), np.float32)
    for c in range(NCORES):
        b, h = divmod(c, 2)
        comb = res.results[c]["comb"]
        out = res.results[c]["out"]
        if h == 1:
            comb = comb[:, roll]
        combined[b, h * QH : (h + 1) * QH] = comb
        output[b, h * QH : (h + 1) * QH] = out
    return output, combined
